# revision 77
# baseline (speedup 1.0000x reference)
"""Trainium2 Bass kernel for nn_CrossAttnBlockppTwoCams.

Sharding: 8 cores = 4 scene-groups x 2 head-halves. Core (g, s) handles scene
group g (batch entries 4g..4g+3) and heads {2s, 2s+1} of all 6 cross-camera
attention pairs -> 12 attention units per core. Each core emits 6 per-pair
partial accumulators (post-W3); the host sums them into the 4 output entries
of the group (each entry receives 1.5 pairs' worth of channels).

Device pipeline per core (all engines balanced, Act-paced ~8.3us/unit):
  - GroupNorm (bf16 x): sum on DVE, sum-of-squares on Act (Square+accum_out),
    group reduce/broadcast via tiny matmuls; rstd = sqrt(recip(var+eps)) so
    only the sqrt and exp activation tables are ever loaded (2 table loads).
  - NIN projections: bf16 matmuls; the 32-channel cond contraction and the
    biases ride in the same PSUM accumulation via 33-partition padded tiles
    (ones row -> bias). V is projected directly in transposed (key-major)
    layout by swapping matmul roles, so no PE transposes. PSUM drains are
    plain copies spread over DVE and Act.
  - Attention unit: scores = k^T q (bf16, 8x[128,1024] PSUM), exp on Act
    (bf16 out; the dominant load: 96 x 1024-col instrs), softmax denominator
    via bf16 pairwise folds (DVE + 2 on GPSIMD) and gpsimd.partition_all_reduce
    (fused partition-reduce+broadcast, SBUF only), reciprocal on DVE,
    normalize fused into the AV-PSUM drain, W3 accumulated in PSUM per pair.
  - Software pipelining: scores of unit u interleave (in the PE stream) with
    AV chunks of unit u-1; NIN for cams 2/3 is deferred into attention slots
    0-5 (using the acc/po PSUM bufs, which W3_SCHED keeps free there); the
    last unit uses a serial fold chain + early AV so the tail is short.

Hardware-verified constraints honored here: GPSIMD cannot touch PSUM; no
DMA to/from PSUM; no f32r x bf16 matmul mixing; no DVE divide; at most one
PSUM operand per vector op. TimelineSim: 149758 ns (baseline was 243824).
"""
import sys

sys.path.insert(0, '/opt/trn_rl_repo')

import numpy as np
import ml_dtypes

B, C, HH, WW = 16, 128, 32, 32
HW = HH * WW
NH, COND, GROUPS, EPS = 4, 32, 32, 1e-6
SCALE = float(C) ** -0.5
PAIRS = [(0, 1), (1, 0), (2, 3), (3, 2), (0, 2), (2, 0)]  # (q cam, kv cam)
BF = ml_dtypes.bfloat16

_PROG = None


def _build_nc():
    import concourse.bacc as bacc
    import concourse.tile as tile
    import concourse.mybir as mybir
    import concourse.bass_isa as bass_isa

    f32 = mybir.dt.float32
    f32r = mybir.dt.float32r
    bf16 = mybir.dt.bfloat16
    AF = mybir.ActivationFunctionType
    ALU = mybir.AluOpType
    X_AX = mybir.AxisListType.X

    nc = bacc.Bacc("TRN2", target_bir_lowering=False, debug=False, num_devices=8)

    d_x = nc.dram_tensor("x4", [4, C, HW], bf16, kind="ExternalInput")
    d_qcp = nc.dram_tensor("qcp", [4, COND + 1, HW], bf16, kind="ExternalInput")
    d_wqkc = nc.dram_tensor("wqkc", [COND + 1, 512], bf16, kind="ExternalInput")
    d_kcp = nc.dram_tensor("kcp", [4, COND + 1, HW], bf16, kind="ExternalInput")
    d_w2c = nc.dram_tensor("w2c", [COND + 1, 256], bf16, kind="ExternalInput")
    d_wqk = nc.dram_tensor("wqk", [C, 512], bf16, kind="ExternalInput")
    d_w2 = nc.dram_tensor("w2", [C, 256], bf16, kind="ExternalInput")
    d_w3 = nc.dram_tensor("w3", [C, 12 * C], bf16, kind="ExternalInput")
    d_gnv = nc.dram_tensor("gnv", [C, 2], f32, kind="ExternalInput")
    d_gind = nc.dram_tensor("gind", [C, GROUPS], bf16, kind="ExternalInput")
    d_gindT = nc.dram_tensor("gindT", [GROUPS, C], bf16, kind="ExternalInput")
    d_out = nc.dram_tensor("out", [6, C, HW], f32, kind="ExternalOutput")

    with tile.TileContext(nc) as tc, nc.allow_low_precision(reason="bf16 pipeline"):
        import contextlib
        ctx = contextlib.ExitStack()
        with ctx:
            cpool = ctx.enter_context(tc.tile_pool(name="consts", bufs=1))
            xpool = ctx.enter_context(tc.tile_pool(name="xp", bufs=1))
            sqpool = ctx.enter_context(tc.tile_pool(name="sqp", bufs=2))
            stpool = ctx.enter_context(tc.tile_pool(name="stp", bufs=2))
            smpool = ctx.enter_context(tc.tile_pool(name="smp", bufs=2))
            sbpool = ctx.enter_context(tc.tile_pool(name="sbp", bufs=2))
            hpool = ctx.enter_context(tc.tile_pool(name="hp", bufs=1))
            qkpool = ctx.enter_context(tc.tile_pool(name="qkp", bufs=1))
            vtpool = ctx.enter_context(tc.tile_pool(name="vtp", bufs=1))
            epool = ctx.enter_context(tc.tile_pool(name="ep", bufs=20))
            fpool = ctx.enter_context(tc.tile_pool(name="fp", bufs=9))
            dpool = ctx.enter_context(tc.tile_pool(name="dp", bufs=2))
            opool = ctx.enter_context(tc.tile_pool(name="op", bufs=6))
            apool = ctx.enter_context(tc.tile_pool(name="ap", bufs=2))
            P1 = ctx.enter_context(tc.tile_pool(name="ps1", bufs=2, space="PSUM"))
            PB = ctx.enter_context(tc.tile_pool(name="psb", bufs=1, space="PSUM"))
            PA = ctx.enter_context(tc.tile_pool(name="psa", bufs=1, space="PSUM"))

            # ---- constants (x + GN path first; w3 last) ----
            xt = [None] * 4
            for m in range(4):
                xt[m] = xpool.tile([C, HW], bf16, tag=f"xt{m}", name=f"xt{m}")
                nc.sync.dma_start(xt[m][:], d_x[m])
            gnv = cpool.tile([C, 2], f32, tag="gnv")
            nc.sync.dma_start(gnv[:], d_gnv[:])
            gind = cpool.tile([C, GROUPS], bf16, tag="gind")
            nc.sync.dma_start(gind[:], d_gind[:])
            gindT = cpool.tile([GROUPS, C], bf16, tag="gindT")
            nc.sync.dma_start(gindT[:], d_gindT[:])
            wqk = cpool.tile([C, 512], bf16, tag="wqk")
            nc.sync.dma_start(wqk[:], d_wqk[:])
            wqkc = cpool.tile([COND + 1, 512], bf16, tag="wqkc")
            nc.sync.dma_start(wqkc[:], d_wqkc[:])
            w2 = cpool.tile([C, 256], bf16, tag="w2")
            nc.sync.dma_start(w2[:], d_w2[:])
            w2c = cpool.tile([COND + 1, 256], bf16, tag="w2c")
            nc.sync.dma_start(w2c[:], d_w2c[:])
            kcp = [None] * 4
            qcp = [None] * 4
            for m in range(4):
                kcp[m] = cpool.tile([COND + 1, HW], bf16, tag=f"kcp{m}", name=f"kcp{m}")
                nc.sync.dma_start(kcp[m][:], d_kcp[m])
                qcp[m] = cpool.tile([COND + 1, HW], bf16, tag=f"qcp{m}", name=f"qcp{m}")
                nc.sync.dma_start(qcp[m][:], d_qcp[m])
            epst = cpool.tile([GROUPS, 1], f32, tag="epst")
            nc.vector.memset(epst[:], EPS)
            w3 = cpool.tile([C, 12 * C], bf16, tag="w3")
            nc.sync.dma_start(w3[:], d_w3[:])

            # GN small PSUM lives in one acc-shaped tile (regions), freed
            # before the attention-phase acc allocations cycle the same buf.
            gn_ps = PA.tile([C, HW], f32, tag="acc", name="gn_ps")

            # GN per cam: sums on the idle Act engine (Identity/Square +
            # accum_out), rstd = sqrt(1/(var+eps)) so only the sqrt+exp act
            # tables are ever loaded. Emission is per-cam so cam0's chain
            # finishes as early as possible.
            h_t = [None] * 4

            def emit_gn(m):
                st = stpool.tile([C, 2], bf16, tag="st", name=f"st{m}", bufs=4)
                nc.vector.tensor_reduce(out=st[:, 0:1], in_=xt[m][:], axis=X_AX, op=ALU.add)
                scr2 = sqpool.tile([C, HW], bf16, tag="sq")
                nc.scalar.activation(scr2[:], xt[m][:], AF.Square, accum_out=st[:, 1:2])
                nc.tensor.matmul(gn_ps[0:GROUPS, 2 * m:2 * m + 2], gind[:], st[:],
                                 start=True, stop=True)
                mu = smpool.tile([GROUPS, 1], f32, tag="mu")
                nc.vector.tensor_copy(mu[:], gn_ps[0:GROUPS, 2 * m:2 * m + 1])
                mu2 = smpool.tile([GROUPS, 1], f32, tag="mu2")
                nc.vector.tensor_tensor(out=mu2[:], in0=mu[:], in1=mu[:], op=ALU.mult)
                vpe = smpool.tile([GROUPS, 1], f32, tag="vpe")
                nc.vector.tensor_tensor(out=vpe[:], in0=gn_ps[0:GROUPS, 2 * m + 1:2 * m + 2],
                                        in1=mu2[:], op=ALU.subtract)
                rv = smpool.tile([GROUPS, 1], f32, tag="rv")
                nc.vector.tensor_scalar_add(out=rv[:], in0=vpe[:], scalar1=epst[:])
                nc.vector.reciprocal(out=rv[:], in_=rv[:])
                rstd = smpool.tile([GROUPS, 1], f32, tag="rstd")
                nc.scalar.activation(rstd[:], rv[:], AF.Sqrt)
                bc_in = smpool.tile([GROUPS, 2], bf16, tag="bcin")
                nc.vector.tensor_copy(bc_in[:, 0:1], rstd[:])
                nc.vector.tensor_copy(bc_in[:, 1:2], mu[:])
                nc.tensor.matmul(gn_ps[:, 16 + 2 * m:18 + 2 * m], gindT[:], bc_in[:],
                                 start=True, stop=True)
                se = sbpool.tile([C, 1], f32, tag="se")
                nc.vector.tensor_tensor(out=se[:], in0=gn_ps[:, 16 + 2 * m:17 + 2 * m],
                                        in1=gnv[:, 0:1], op=ALU.mult)
                ms = smpool.tile([C, 1], f32, tag="ms")
                nc.vector.tensor_tensor(out=ms[:], in0=gn_ps[:, 17 + 2 * m:18 + 2 * m],
                                        in1=se[:], op=ALU.mult)
                be = sbpool.tile([C, 1], f32, tag="be")
                nc.vector.tensor_tensor(out=be[:], in0=gnv[:, 1:2], in1=ms[:], op=ALU.subtract)
                ht = hpool.tile([C, HW], bf16, tag=f"ht{m}", name=f"ht{m}")
                nc.vector.tensor_scalar(out=ht[:], in0=xt[m][:], scalar1=se[:], scalar2=be[:],
                                        op0=ALU.mult, op1=ALU.add)
                h_t[m] = ht

            for _m in range(4):
                emit_gn(_m)

            q_sb = [[None] * 2 for _ in range(4)]
            k_sb = [[None] * 2 for _ in range(4)]
            vt_sb = [[None] * 2 for _ in range(4)]

            def emit_nin_qk(m, proj, i, pool, tagn, eng, defer=False):
                ht = h_t[m]
                cp = qcp[m] if proj == 0 else kcp[m]
                ps = pool.tile([C, HW], f32, tag=tagn, name="ps_nin")
                wblk = wqk[:, (proj * 2 + i) * 128:(proj * 2 + i + 1) * 128]
                wcblk = wqkc[:, (proj * 2 + i) * 128:(proj * 2 + i + 1) * 128]
                for hf in range(2):
                    fr = slice(hf * 512, (hf + 1) * 512)
                    nc.tensor.matmul(ps[:, fr], wblk, ht[:, fr], start=True, stop=False)
                    nc.tensor.matmul(ps[:, fr], wcblk, cp[:, fr], start=False, stop=True)
                t = qkpool.tile([C, HW], bf16, tag=f"qk{m}_{proj}_{i}",
                                name=f"qk{m}_{proj}_{i}")

                def drain():
                    if eng == 'act':
                        nc.scalar.activation(t[:], ps[:], AF.Identity)
                    else:
                        nc.vector.tensor_copy(t[:], ps[:])
                (q_sb if proj == 0 else k_sb)[m][i] = t
                if defer:
                    return drain
                drain()

            def emit_nin_vt(m, i, pool, tagn, vt_eng, defer=False):
                # vT NIN: transposed roles -> output lands key-partitioned.
                # Cond+bias contraction also on the PE (33-partition padded),
                # so the drain is a plain copy.
                ht = h_t[m]
                ps = pool.tile([128, HW], f32, tag=tagn, name="ps_vt")
                for blk in range(8):
                    fr = slice(blk * 128, (blk + 1) * 128)
                    nc.tensor.matmul(ps[:, fr], ht[:, fr], w2[:, i * 128:(i + 1) * 128],
                                     start=True, stop=False)
                    nc.tensor.matmul(ps[:, fr], kcp[m][:, fr], w2c[:, i * 128:(i + 1) * 128],
                                     start=False, stop=True)
                vt = vtpool.tile([128, HW], bf16, tag=f"vt{m}_{i}", name=f"vt{m}_{i}")

                def drain():
                    if vt_eng == 'act':
                        nc.scalar.activation(vt[:], ps[:], AF.Identity)
                    else:
                        nc.vector.tensor_copy(vt[:], ps[:])
                vt_sb[m][i] = vt
                if defer:
                    return drain
                drain()

            # cams 0/1 up front (prologue), rotating 3 PSUM bufs; vt drains on
            # the idle Act engine.
            rot = [(PB, "po"), (P1, "mm"), (P1, "mm")]
            nn = 0
            for (mq, mk) in ((0, 1), (1, 0)):
                for i in range(2):
                    for kind, mm_, pj in (('qk', mq, 0), ('qk', mk, 1), ('vt', mk, None)):
                        pool, tagn = rot[nn % 3]
                        nn += 1
                        if kind == 'qk':
                            emit_nin_qk(mm_, pj, i, pool, tagn,
                                        'act' if nn % 2 == 0 else 'dve')
                        else:
                            emit_nin_vt(mm_, i, pool, tagn, 'act' if mm_ == 1 else 'dve')

            # cams 2/3: deferred into attention slots 0-3 (PA + po PSUM slots,
            # which are free until the first W3 closes at slot 4). Ordered by
            # first use: pair 2 needs (c2 q, c3 k/vt); pair 3 the reverse.
            deferred = []
            for (mq, mk) in ((2, 3), (3, 2)):
                for i in range(2):
                    deferred.append(lambda pool, tagn, mq=mq, i=i:
                                    emit_nin_qk(mq, 0, i, pool, tagn, 'act'))
                    deferred.append(lambda pool, tagn, mk=mk, i=i:
                                    emit_nin_qk(mk, 1, i, pool, tagn, 'dve'))
                    deferred.append(lambda pool, tagn, mk=mk, i=i:
                                    emit_nin_vt(mk, i, pool, tagn, 'act'))

            # ---- attention: 6 pairs x 2 heads, software-pipelined ----
            # Per slot u: scores/exp/folds of unit u interleaved (on the PE
            # stream) with AV chunks of unit u-1, so the PE fills the gaps
            # while Act paces the pipeline. W3+accumulate+drain close per
            # PAIR on a schedule that keeps the single acc PSUM buf free
            # during slots 0-3 (used by the deferred NIN).
            osb = {}

            def close_pair(pr, last_osb=None, drain_eng='dve'):
                accp = PA.tile([C, HW], f32, tag="acc", name=f"acc{pr}")
                for uu in (2 * pr, 2 * pr + 1):
                    ob = osb[uu] if last_osb is None or uu != 2 * pr + 1 else last_osb
                    w3u = w3[:, uu * 128:(uu + 1) * 128]
                    st, sp = uu == 2 * pr, uu == 2 * pr + 1
                    nc.tensor.matmul(accp[:, 0:512], w3u, ob[:, 0:512], start=st, stop=sp,
                                     skip_group_check=True)
                    nc.tensor.matmul(accp[:, 512:1024], w3u, ob[:, 512:1024], start=st, stop=sp,
                                     skip_group_check=True)
                asb = apool.tile([C, HW], f32, tag="asb", name=f"asb{pr}")
                if drain_eng == 'act':
                    nc.scalar.activation(asb[:], accp[:], AF.Identity)
                else:
                    nc.vector.tensor_copy(asb[:], accp[:])
                nc.sync.dma_start(d_out[pr], asb[:])

            def emit_slot(u, prev, hooks, w3_pairs, self_tail=False):
                p, i = u // 2, u % 2
                qc, kc = PAIRS[p]
                qs, ks = q_sb[qc][i], k_sb[kc][i]
                if prev is not None:
                    pu, pE, pdbc = prev
                    pvts = vt_sb[PAIRS[pu // 2][1]][pu % 2]
                    ps_o = PB.tile([C, HW], f32, tag="po", name="ps_o")

                def av_chunk(k0, k1):
                    if prev is None:
                        return
                    for kt in range(k0, k1):
                        st, sp = kt == 0, kt == 7
                        lhs = pvts[:, kt * 128:(kt + 1) * 128]
                        nc.tensor.matmul(ps_o[:, 0:512], lhs, pE[kt][:, 0:512], start=st, stop=sp)
                        nc.tensor.matmul(ps_o[:, 512:1024], lhs, pE[kt][:, 512:1024],
                                         start=st, stop=sp)

                E = []
                fs = {}

                def sc(kt):
                    ps_s = P1.tile([C, HW], f32, tag="mm", name="ps_s")
                    lhs = ks[:, kt * 128:(kt + 1) * 128]
                    nc.tensor.matmul(ps_s[:, 0:512], lhs, qs[:, 0:512], start=True, stop=True)
                    nc.tensor.matmul(ps_s[:, 512:1024], lhs, qs[:, 512:1024], start=True, stop=True)
                    e_t = epool.tile([C, HW], bf16, tag="et")
                    nc.scalar.activation(e_t[:], ps_s[:], AF.Exp, scale=SCALE)
                    E.append(e_t)

                def fold(a, b, dst=None, eng=None):
                    e = eng or nc.vector
                    if dst is None:
                        dst = fpool.tile([C, HW], bf16, tag="f", name="fold")
                        e.tensor_tensor(out=dst[:], in0=a[:], in1=b[:], op=ALU.add)
                    else:
                        e.tensor_tensor(out=dst[:], in0=dst[:], in1=a[:], op=ALU.add)
                    return dst

                drains = []
                sc(0)
                sc(1)
                fs['f01'] = fold(E[0], E[1])
                if len(hooks) > 0:
                    drains.append(hooks[0](PA, "acc"))
                av_chunk(0, 3)
                sc(2)
                if self_tail:
                    fold(E[2], None, dst=fs['f01'])
                sc(3)
                if self_tail:
                    fold(E[3], None, dst=fs['f01'])
                else:
                    fs['f23'] = fold(E[2], E[3], eng=nc.gpsimd)
                    fold(fs['f23'], None, dst=fs['f01'])
                av_chunk(3, 6)
                sc(4)
                if self_tail:
                    fold(E[4], None, dst=fs['f01'])
                sc(5)
                if self_tail:
                    fold(E[5], None, dst=fs['f01'])
                else:
                    fs['f45'] = fold(E[4], E[5], eng=nc.gpsimd)
                av_chunk(6, 8)
                o_sb = None
                if prev is not None:
                    o_sb = opool.tile([C, HW], bf16, tag="osb", name="o_sb")
                    nc.vector.tensor_tensor(out=o_sb[:], in0=ps_o[:], in1=pdbc[:], op=ALU.mult)
                    osb[pu] = o_sb
                if len(hooks) > 1:
                    drains.append(hooks[1](PB, "po"))
                if self_tail:
                    ps_os = PB.tile([C, HW], f32, tag="po", name="ps_os")

                    def av_self(k0, k1):
                        for kt in range(k0, k1):
                            st, sp = kt == 0, kt == 7
                            lhs = vt_sb[kc][i][:, kt * 128:(kt + 1) * 128]
                            nc.tensor.matmul(ps_os[:, 0:512], lhs, E[kt][:, 0:512],
                                             start=st, stop=sp)
                            nc.tensor.matmul(ps_os[:, 512:1024], lhs, E[kt][:, 512:1024],
                                             start=st, stop=sp)
                    av_self(0, 4)
                sc(6)
                if self_tail:
                    fold(E[6], None, dst=fs['f01'])
                    av_self(4, 6)
                sc(7)
                if self_tail:
                    fold(E[7], None, dst=fs['f01'])
                else:
                    fs['f67'] = fold(E[6], E[7])
                    fold(fs['f67'], None, dst=fs['f45'])
                    fold(fs['f45'], None, dst=fs['f01'])
                dbc = dpool.tile([C, HW], f32, tag="dbc", name="dbc")
                nc.gpsimd.partition_all_reduce(dbc[:], fs['f01'][:], channels=128,
                                               reduce_op=bass_isa.ReduceOp.add)
                nc.vector.reciprocal(out=dbc[:], in_=dbc[:])
                for dr in drains:
                    if dr is not None:
                        dr()
                if len(hooks) > 2:
                    hooks[2](PA, "acc")
                for pr in w3_pairs:
                    close_pair(pr)
                if self_tail:
                    av_self(6, 8)
                    o_sbs = opool.tile([C, HW], bf16, tag="osb", name="o_sbs")
                    nc.vector.tensor_tensor(out=o_sbs[:], in0=ps_os[:], in1=dbc[:], op=ALU.mult)
                    close_pair(5, last_osb=o_sbs, drain_eng='act')
                return E, dbc

            W3_SCHED = {3: [0], 5: [1], 6: [2], 8: [3], 10: [4]}
            pend = None
            for u in range(12):
                hooks = deferred[2 * u:2 * u + 2] if u < 6 else []
                E, dbc = emit_slot(u, pend, hooks, W3_SCHED.get(u, []),
                                   self_tail=(u == 11))
                pend = (u, E, dbc)

    nc.compile()
    return nc


def _get_prog():
    global _PROG
    if _PROG is None:
        _PROG = _build_nc()
    return _PROG


def _pack_host(x, q_cond, k_a_cond, k_b_cond, gn_scale, gn_bias,
               W0, b0, W1, b1, W2, b2, W3, b3):
    f4 = np.float32
    x = np.ascontiguousarray(x, f4).reshape(B, C, HW)
    q_cs = np.repeat(np.ascontiguousarray(q_cond, f4).reshape(B // 2, COND, HW), 2, axis=0)
    k_cs = np.stack([np.ascontiguousarray(k_a_cond, f4).reshape(B // 2, COND, HW),
                     np.ascontiguousarray(k_b_cond, f4).reshape(B // 2, COND, HW)],
                    axis=1).reshape(B, COND, HW)


    gind = np.zeros((C, GROUPS), f4)
    for c in range(C):
        gind[c, c // (C // GROUPS)] = 1.0 / (C // GROUPS * HW)
    gindT = np.zeros((GROUPS, C), f4)
    for c in range(C):
        gindT[c // (C // GROUPS), c] = 1.0
    gnv = np.stack([np.asarray(gn_scale, f4), np.asarray(gn_bias, f4)], axis=1)

    in_maps = []
    for core in range(8):
        g, s = core // 2, core % 2
        hsel = [2 * s, 2 * s + 1]
        cams = [4 * g + m for m in range(4)]
        x4 = x[cams].astype(BF)
        kcp = np.ones((4, COND + 1, HW), f4)
        kcp[:, :COND] = k_cs[cams]
        qcp = np.ones((4, COND + 1, HW), f4)
        qcp[:, :COND] = q_cs[cams]
        wqkc = np.zeros((COND + 1, 512), f4)
        for i in range(2):
            cl = slice(128 * hsel[i], 128 * hsel[i] + 128)
            wqkc[:COND, i * 128:(i + 1) * 128] = W0[C:, cl]
            wqkc[COND, i * 128:(i + 1) * 128] = b0[cl]
            wqkc[:COND, 256 + i * 128:256 + (i + 1) * 128] = W1[C:, cl]
            wqkc[COND, 256 + i * 128:256 + (i + 1) * 128] = b1[cl]
        w2c = np.zeros((COND + 1, 256), f4)
        for i in range(2):
            w2c[:COND, i * 128:(i + 1) * 128] = W2[C:, 128 * hsel[i]:128 * hsel[i] + 128]
            w2c[COND, i * 128:(i + 1) * 128] = b2[128 * hsel[i]:128 * hsel[i] + 128]
        wqk = np.concatenate([W0[:C, 128 * hsel[0]:128 * hsel[0] + 128],
                              W0[:C, 128 * hsel[1]:128 * hsel[1] + 128],
                              W1[:C, 128 * hsel[0]:128 * hsel[0] + 128],
                              W1[:C, 128 * hsel[1]:128 * hsel[1] + 128]], axis=1).astype(BF)
        w2m = np.concatenate([W2[:C, 128 * hsel[0]:128 * hsel[0] + 128],
                              W2[:C, 128 * hsel[1]:128 * hsel[1] + 128]], axis=1).astype(BF)
        w3l = np.zeros((C, 12 * C), f4)
        for p in range(6):
            for i in range(2):
                u = p * 2 + i
                ch = 512 * p + 128 * hsel[i]
                r = ch % 768
                w3l[:, u * C:(u + 1) * C] = W3[r:r + C, :]
        in_maps.append({
            "x4": x4, "qcp": qcp.astype(BF), "wqkc": wqkc.astype(BF),
            "kcp": kcp.astype(BF), "w2c": w2c.astype(BF),
            "wqk": wqk, "w2": w2m, "w3": w3l.astype(BF),
            "gnv": gnv, "gind": gind.astype(BF), "gindT": gindT.astype(BF),
        })
    return in_maps


def _assemble(results, x, b3):
    x = np.ascontiguousarray(x, np.float32)
    out = x + np.asarray(b3, np.float32)[None, :, None, None]
    for core in range(8):
        g, s = core // 2, core % 2
        o = results[core]["out"].reshape(6, C, HH, WW)
        for p in range(6):
            j = (512 * p + 256 * s) // 768
            out[4 * g + j] += o[p]
    return out


def kernel(**inputs):
    from concourse.bass_utils import run_bass_kernel_spmd
    nc = _get_prog()
    ins = {k: np.asarray(v) for k, v in inputs.items()}
    in_maps = _pack_host(**ins)
    res = run_bass_kernel_spmd(nc, in_maps, core_ids=list(range(8)))
    return _assemble(res.results, ins["x"], ins["b3"])
